# revision 46
# baseline (speedup 1.0000x reference)
"""Trainium2 Bass kernel for nn_Attention_54614804136573 (topk_masking).

Sharding: 8 cores = 4 batches x 2 head-groups (8 heads each). Each core gets
its batch's full x (columns rotated so its own 8 head-chunks come first, in
bf16), computes the token-importance mask redundantly, runs its 8 heads of
attention, and produces a partial to_out product for its 1024-wide d-slice.
The host sums the two partials per batch and adds bo.

Key structural choices (vs the straightforward formulation):
- x arrives bf16 and is transposed by the DMA xbar (dma_start_transpose), so
  the PE array never spends cycles on data-movement transposes.
- K projection is eliminated: scores = x^T (Wq^T Wk) x, with the single
  projected operand qg = (Wq^T Wk)^T-applied to xT. Q/K biases fold away:
  the per-query term is softmax-shift-invariant, the per-key term
  kcol_j = (Wk^T bq).x_j enters the exp() bias operand.
- The top-25 mask is computed with gpsimd kth_largest on the [128, 8]
  column layout; masked tokens get exactly 0 (their softmax value ~1e-3
  contributes ~1e-5 relative, far below tolerance).
- Softmax denominators come from near-free ap=1 matmuls (stationary = pexp
  tile, moving = a ones column), accumulated in PSUM across key tiles.
"""

import sys

sys.path.insert(0, "/opt/trn_rl_repo")

import numpy as np
import ml_dtypes

import concourse.mybir as mybir
import concourse.tile as tile
from concourse import bacc, bass_utils
from concourse.masks import make_identity
from concourse.tile import add_dep_helper

B = 4
N = 1024
C = 128
D = 2048
NCHUNK = 16
HPC = 8  # heads (= 128-wide d-chunks) per core
MASK_NUM = 25
SCALE = 64.0 ** -0.5  # 0.125

F32 = mybir.dt.float32
F32R = mybir.dt.float32r
BF16 = mybir.dt.bfloat16
Exp = mybir.ActivationFunctionType.Exp
Ident = mybir.ActivationFunctionType.Identity
Mult = mybir.AluOpType.mult
IsLt = mybir.AluOpType.is_lt


def _body(tc, xc, wpack_d, bv_d, wo_d, outT_d):
    nc = tc.nc
    import concourse.bass_isa as bass_isa

    with (
        tc.tile_pool(name="consts", bufs=1) as consts,
        tc.tile_pool(name="persist", bufs=1) as persist,
        tc.tile_pool(name="rows", bufs=2) as rows,
        tc.tile_pool(name="vtp", bufs=2) as vtp,
        tc.tile_pool(name="pexp", bufs=4) as pexp,
        tc.tile_pool(name="rbp", bufs=2) as rbp,
        tc.tile_pool(name="fop", bufs=3) as fop,
        tc.tile_pool(name="tiny", bufs=2) as tiny,
        tc.tile_pool(name="bigp", bufs=2, space="PSUM") as bigp,
        tc.tile_pool(name="otp", bufs=1, space="PSUM") as otp,
        tc.tile_pool(name="smp", bufs=1, space="PSUM") as smp,
        tc.tile_pool(name="foap", bufs=1, space="PSUM") as foap,
    ):
        # ---- constants / weights ----
        ident = consts.tile([128, 128], F32)
        make_identity(nc, ident)
        ones_bf = consts.tile([128, 1], BF16)
        nc.vector.memset(ones_bf, 1.0)
        # all small weights packed into one DMA (HWDGE sem lanes are a
        # scarce resource early on; see tile_sem_assignment round-robin).
        # bf16 throughout: walrus rejects mixed 32/16-bit matmul operands.
        wpack_sb = consts.tile([C, 258], BF16)
        nc.sync.dma_start(out=wpack_sb, in_=wpack_d)
        g_sb = wpack_sb[:, 0:128]
        wvT_sb = wpack_sb[:, 128:256]
        wtc_sb = wpack_sb[:, 256:257]
        w2_sb = wpack_sb[:, 257:258]
        bv_sb = consts.tile([C, 1], F32)
        nc.sync.dma_start(out=bv_sb, in_=bv_d)
        # warm the exp activation table while everything else loads
        junk = consts.tile([128, 8], F32)
        nc.vector.memset(junk, 0.0)
        nc.scalar.activation(out=junk, in_=junk, func=Exp)

        # ---- persistent activations ----
        GRP0 = [0, 2, 6, 10, 14, 16]
        xTv = [
            persist.tile([128, GRP0[i + 1] - GRP0[i], N], BF16,
                         name=f"xTg{i}")
            for i in range(5)
        ]  # [c, k-in-group, n]

        def xT(k):
            gi = max(i for i in range(5) if GRP0[i] <= k)
            return xTv[gi][:, k - GRP0[gi], :]
        qg = persist.tile([128, HPC, N], BF16)  # [c, h, i]
        foA = persist.tile([128, 16, N], BF16)  # to_out partial, heads 0-3
        vnat = [
            persist.tile([128, 8, C], BF16, name=f"vnat{h}") for h in range(HPC)
        ]  # per head: [j, jt, c]
        woT_sb = persist.tile([128, HPC, D], BF16)  # [c-in-chunk, h, oc]
        outT = persist.tile([128, HPC, N], BF16)  # [c, h, i]
        kcol_sb = persist.tile([128, HPC, 8], F32)
        ebias = persist.tile([128, HPC, 8], F32)
        neglg = persist.tile([128, 8], F32)
        thr2 = persist.tile([1, 2], F32)
        thrb = persist.tile([128, 1], F32)
        scale_col = persist.tile([128, 8], F32)

        # ---- x load+transpose via DMA xbar ----
        # group sizes [2,4,4,4,2]: a small first group gets the PE started
        # ~2us earlier; totals are unchanged. The first transpose leads the
        # sync queue; the small weight loads ride behind it.
        GRP = [(0, 2), (2, 4), (6, 4), (10, 4), (14, 2)]
        for gi, (k0, nk) in enumerate(GRP):
            nc.sync.dma_start_transpose(
                xTv[gi][:, :, :],
                xc[:, k0 * 128 : (k0 + nk) * 128],
            )

        # ---- logits, directly in [token%128, token//128] column layout:
        # lgcol[:, t] += xT(k)[:, t-block]^T @ wtc  (ap=1 matmuls, ~free) ----
        lg = otp.tile([128, N], F32, tag="ot")
        lgcol = lg[:, 0:8]

        def lg_chunks(ks, last=False):
            for k in ks:
                for t in range(8):
                    # start only once per PSUM bank: start_tensor_calc
                    # zeroes the whole 2KB zero-region, so later column
                    # groups must rely on the pending-zero first-write
                    nc.tensor.matmul(
                        lgcol[:, t : t + 1],
                        xT(k)[:, t * 128 : (t + 1) * 128],
                        wtc_sb,
                        start=(k == 0 and t == 0),
                        stop=(last and k == ks[-1]),
                    )

        # kcol[j] = (Wk^T bq) . x_j per head, directly in column layout
        def emit_kcol(hs):
            for h in hs:
                kc = smp.tile([128, 8], F32, tag="sm", name=f"kc{h}")
                for jt in range(8):
                    nc.tensor.matmul(
                        kc[:, jt : jt + 1],
                        xT(h)[:, jt * 128 : (jt + 1) * 128],
                        w2_sb,
                        start=(jt == 0),
                        stop=True,
                    )
                nc.vector.tensor_copy(kcol_sb[:, h, :], kc)

        # qg / vT projections, interleaved with the tail logits chunks
        def emit_qg(h):
            pp = bigp.tile([128, N], F32, tag="big")
            for half in range(2):
                nc.tensor.matmul(
                    pp[:, half * 512 : (half + 1) * 512],
                    g_sb,
                    xT(h)[:, half * 512 : (half + 1) * 512],
                    start=True,
                    stop=True,
                )
            nc.vector.tensor_copy(qg[:, h, :], pp)

        def emit_vt(h):
            pp = bigp.tile([128, N], F32, tag="big")
            for half in range(2):
                nc.tensor.matmul(
                    pp[:, half * 512 : (half + 1) * 512],
                    wvT_sb,
                    xT(h)[:, half * 512 : (half + 1) * 512],
                    start=True,
                    stop=True,
                )
            vt_row = vtp.tile([128, N], BF16)
            # phase-1 heads bias on Act (done before the exps own it);
            # phase-2-deferred heads bias on DVE
            if h < 4:
                nc.scalar.activation(out=vt_row, in_=pp, func=Ident,
                                     bias=bv_sb)
            else:
                nc.vector.tensor_scalar_add(vt_row, pp, bv_sb)
            nc.sync.dma_start_transpose(vnat[h][:, :, :], vt_row)
            if h >= 4:
                # deferred heads: the mask multiply must follow the transpose
                nc.vector.tensor_tensor(
                    out=vnat[h][:, :, :],
                    in0=vnat[h][:, :, :],
                    in1=scale_col.unsqueeze(-1).broadcast_to([128, 8, C]),
                    op=Mult,
                )

        # Phase-1 ordering principles: (1) every vt bias-copy must clear
        # the Act engine before the mask lands (the exps own Act from then
        # on); (2) the logits tail chunks are emitted with nothing
        # PSUM-slot-blocked in front of them, since the mask gates the
        # whole attention phase; (3) heads 4-7 qg projections migrate into
        # the activation-bound early attention steps.
        lg_chunks([0, 1])
        emit_kcol([0, 1])
        emit_qg(0)
        emit_vt(0)
        emit_qg(1)
        emit_vt(1)
        lg_chunks([2, 3, 4, 5])
        emit_kcol([2, 3, 4, 5])
        emit_qg(2)
        emit_vt(2)
        emit_qg(3)
        emit_vt(3)
        lg_chunks([6, 7])
        emit_kcol([6, 7])
        lg_chunks([8, 9])
        lg_chunks([10, 11])
        lg_chunks([12, 13])
        lg_chunks([14, 15], last=True)

        # ---- mask from the (host-negated) logit columns ----
        # lgcol holds -logits (wtc is negated on the host), so the bottom-25
        # threshold is the midpoint of its 25th/26th largest values
        nc.vector.tensor_copy(neglg, lgcol)
        nc.gpsimd.kth_largest(
            thr2, neglg, 8, MASK_NUM, quantile=1.0 - 24.5 / (N - 1.0)
        )
        nc.gpsimd.partition_broadcast(thrb, thr2[0:1, 0:1], 128)
        nc.vector.tensor_scalar(
            scale_col, neglg, thrb[:, 0:1], SCALE, op0=IsLt, op1=Mult
        )
        for h in range(HPC):
            nc.vector.tensor_tensor(
                out=ebias[:, h, :], in0=kcol_sb[:, h, :], in1=scale_col, op=Mult
            )
        # mask the value tiles (per-key-token = per-partition in vnat
        # layout); heads 4-7 are masked inside their deferred emit_vt
        for h in range(4):
            nc.vector.tensor_tensor(
                out=vnat[h][:, :, :],
                in0=vnat[h][:, :, :],
                in1=scale_col.unsqueeze(-1).broadcast_to([128, 8, C]),
                op=Mult,
            )

        # ================= phase 2: attention ==============================
        # Flattened (h, jt) software pipeline: PV/dens for step k are
        # emitted after ST/exp of step k+1, so the PE never waits on the
        # activation engine at head boundaries.
        heads = {}

        def start_head(h):
            ot_t = otp.tile([128, N], F32, tag="ot", name=f"ot{h}")
            # dn occupies the first 8 columns; the recip-transpose target
            # lives in the same bank at [0:8, 8:136]
            dn_t = smp.tile([128, 136], F32, tag="sm", name=f"dn{h}")
            heads[h] = (ot_t, dn_t)

        def emit_pv_dens(h, jt, pexp_t):
            ot, dnt = heads[h]
            dn = dnt[:, 0:8]
            for half in range(2):
                nc.tensor.matmul(
                    ot[:, half * 512 : (half + 1) * 512],
                    vnat[h][:, jt, :],
                    pexp_t[:, half * 512 : (half + 1) * 512],
                    start=(jt == 0),
                    stop=(jt == 7),
                )
            for ib in range(8):
                nc.tensor.matmul(
                    dn[:, ib : ib + 1],
                    pexp_t[:, ib * 128 : (ib + 1) * 128],
                    ones_bf,
                    start=(jt == 0 and ib == 0),
                    stop=(jt == 7),
                )
            if jt == 7:
                finish_head(h)
            # stream the heads-0..3 part of to_out through the exp-bound
            # window of heads 4..7 (one [128,512] tile per pipeline step)
            if h >= 4 and (h, jt) >= (4, 2):
                step = (h - 4) * 8 + jt - 2
                for fi in ([step] if step < 28 else [2 * step - 28,
                                                     2 * step - 27]):
                    oc, sh = divmod(fi, 2)
                    foa = foap.tile([128, 512], F32, tag="foa",
                                    name=f"foa{fi}")
                    for hp in range(4):
                        nc.tensor.matmul(
                            foa,
                            woT_sb[:, hp, oc * 128 : (oc + 1) * 128],
                            outT[:, hp, sh * 512 : (sh + 1) * 512],
                            start=(hp == 0),
                            stop=(hp == 3),
                        )
                    nc.vector.tensor_copy(
                        foA[:, oc, sh * 512 : (sh + 1) * 512], foa
                    )

        def finish_head(h):
            # dens columns -> reciprocal -> row -> broadcast; the raw
            # (unnormalized) PV result is copied out immediately so the ot
            # PSUM frees for the next head, then normalized in place.
            ot, dnt = heads.pop(h)
            recip_sb = tiny.tile([128, 8], F32)
            nc.vector.reciprocal(recip_sb, dnt[:, 0:8])
            rt = dnt[0:8, 8:136]
            nc.tensor.transpose(rt, recip_sb, ident)
            nc.vector.tensor_copy(outT[:, h, :], ot)
            rt_sb = tiny.tile([8, 128], F32, tag="rt")
            nc.vector.tensor_copy(rt_sb, rt)
            rrow = rows.tile([1, N], F32)
            nc.sync.dma_start(out=rrow, in_=rt_sb)
            rb = rbp.tile([128, N], F32)
            nc.gpsimd.partition_broadcast(rb, rrow, 128)
            nc.vector.tensor_tensor(
                out=outT[:, h, :], in0=outT[:, h, :], in1=rb, op=Mult
            )

        pending = None
        for idx in range(HPC * 8):
            h, jt = divmod(idx, 8)
            if jt == 0:
                start_head(h)
            st = bigp.tile([128, N], F32, tag="big")
            for half in range(2):
                nc.tensor.matmul(
                    st[:, half * 512 : (half + 1) * 512],
                    xT(h)[:, jt * 128 : (jt + 1) * 128],
                    qg[:, h, half * 512 : (half + 1) * 512],
                    start=True,
                    stop=True,
                )
            pexp_t = pexp.tile([128, N], BF16)
            exp_i = nc.scalar.activation(
                out=pexp_t,
                in_=st,
                func=Exp,
                scale=scale_col[:, jt : jt + 1],
                bias=ebias[:, h, jt : jt + 1],
            )
            if jt == 4 and h < 4:
                # heads 4-7 score projections, hidden in the exp-bound window
                emit_qg(h + 4)
            if jt == 6 and h < 4:
                emit_vt(h + 4)
            if jt == 0:
                # stream one woT chunk per head; the explicit dep on the
                # head's first exp keeps the scheduler from hoisting these
                # bulk loads in front of the critical x transposes and
                # mask round-trips on the shared DMA engines
                wo_i = nc.gpsimd.dma_start(
                    out=woT_sb[:, h, :], in_=wo_d[h * 128 : (h + 1) * 128, :]
                )
                add_dep_helper(
                    wo_i.ins, exp_i.ins, sync=True, reason="defer woT load"
                )
            if pending is not None:
                emit_pv_dens(*pending)
            pending = (h, jt, pexp_t)
        emit_pv_dens(*pending)

        # ================= phase 3: to_out partial =========================
        def finish_oc(oc, fo):
            # bf16 output halves the writeback; pieces keep the tail DMA
            # from waiting on the full-row add (finer near the end)
            fout = fop.tile([128, N], BF16)
            npc = 2
            w = N // npc
            for sh in range(npc):
                nc.vector.tensor_tensor(
                    out=fout[:, sh * w : (sh + 1) * w],
                    in0=fo[:, sh * w : (sh + 1) * w],
                    in1=foA[:, oc, sh * w : (sh + 1) * w],
                    op=mybir.AluOpType.add,
                )
                eng = nc.sync if sh % 2 == 0 else nc.scalar
                eng.dma_start(
                    out=outT_d[oc * 128 : (oc + 1) * 128, sh * w : (sh + 1) * w],
                    in_=fout[:, sh * w : (sh + 1) * w],
                )

        pending_oc = None
        for oc in range(16):
            # rotate through the big pool AND the (now idle) ot bank for a
            # 3-deep accumulation pipeline in the final projection
            if oc % 3 == 2:
                fo = otp.tile([128, N], F32, tag="ot", name=f"foB{oc}")
            else:
                fo = bigp.tile([128, N], F32, tag="big", name=f"foB{oc}")
            # heads 4..7 only (0..3 were accumulated into foA during
            # phase 2); h outer so the last head's operand is needed last
            for h in range(4, HPC):
                for half in range(2):
                    nc.tensor.matmul(
                        fo[:, half * 512 : (half + 1) * 512],
                        woT_sb[:, h, oc * 128 : (oc + 1) * 128],
                        outT[:, h, half * 512 : (half + 1) * 512],
                        start=(h == 4),
                        stop=(h == HPC - 1),
                    )
            if pending_oc is not None:
                finish_oc(*pending_oc)
            pending_oc = (oc, fo)
        finish_oc(*pending_oc)


_CACHE = {}


def _get_module():
    if "nc" in _CACHE:
        return _CACHE["nc"]
    nc = bacc.Bacc("TRN2", target_bir_lowering=False, debug=False, num_devices=8)
    xc = nc.dram_tensor("xc", (N, D), BF16, kind="ExternalInput").ap()
    wpack_d = nc.dram_tensor("wpack", (C, 258), BF16, kind="ExternalInput").ap()
    bv_d = nc.dram_tensor("bv", (C, 1), F32, kind="ExternalInput").ap()
    wo_d = nc.dram_tensor("woT", (HPC * C, D), BF16, kind="ExternalInput").ap()
    outT_d = nc.dram_tensor("outT", (D, N), BF16, kind="ExternalOutput").ap()

    with tile.TileContext(nc) as tc:
        _body(tc, xc, wpack_d, bv_d, wo_d, outT_d)
    nc.compile()
    _CACHE["nc"] = nc
    return nc


def make_in_maps(x, Wq, bq, Wk, bk, Wv, bv, Wl, bl, Wo, bo):
    x = np.ascontiguousarray(np.asarray(x, np.float32))
    Wq = np.asarray(Wq, np.float32)
    Wk = np.asarray(Wk, np.float32)
    Wv = np.asarray(Wv, np.float32)
    Wl = np.asarray(Wl, np.float32)
    Wo = np.asarray(Wo, np.float32)
    bq = np.asarray(bq, np.float32)

    gmat = Wq.T @ Wk  # scores = x^T G x
    w2 = (Wk.T @ bq).reshape(C, 1)  # per-key bias column
    # negated so the device-side columns are -logits (mask needs the
    # 25th largest of the negation; saves a pass)
    wtc = (-(Wl[0] @ Wq) / float(NCHUNK)).reshape(C, 1)
    wpack = np.concatenate([gmat, Wv.T, wtc, w2], axis=1)
    common = {
        "wpack": np.ascontiguousarray(wpack).astype(ml_dtypes.bfloat16),
        "bv": np.asarray(bv, np.float32).reshape(C, 1),
    }
    woT = np.ascontiguousarray(Wo.T)  # (d, o)
    # the V-side mask multiply uses mask*SCALE (saves a pass); Wo absorbs
    # the exact power-of-two compensation factor 1/SCALE = 8
    woT_half = [
        np.ascontiguousarray(woT[0:1024, :] * 8.0).astype(ml_dtypes.bfloat16),
        np.ascontiguousarray(woT[1024:2048, :] * 8.0).astype(ml_dtypes.bfloat16),
    ]
    in_maps = []
    for core in range(8):
        b, g = divmod(core, 2)
        xb = x[b]
        xcore = xb if g == 0 else np.concatenate(
            [xb[:, 1024:], xb[:, :1024]], axis=1
        )
        xcore = np.ascontiguousarray(xcore).astype(ml_dtypes.bfloat16)
        in_maps.append({"xc": xcore, "woT": woT_half[g], **common})
    return in_maps


def run_spmd(in_maps, trace=False, **kw):
    nc = _get_module()
    return bass_utils.run_bass_kernel_spmd(
        nc, in_maps, core_ids=list(range(8)), trace=trace, **kw
    )


def gather(results, bo):
    bo = np.asarray(bo, np.float32)
    out = np.empty((B, N, D), np.float32)
    for b in range(B):
        p0 = np.asarray(results[2 * b]["outT"], np.float32).T
        p1 = np.asarray(results[2 * b + 1]["outT"], np.float32).T
        out[b] = p0 + p1 + bo
    return out


def kernel(x, Wq, bq, Wk, bk, Wv, bv, Wl, bl, Wo, bo, stage=None, **_unused):
    in_maps = make_in_maps(x, Wq, bq, Wk, bk, Wv, bv, Wl, bl, Wo, bo)
    try:
        res = run_spmd(in_maps)
    except Exception:
        # transient device/runtime hiccup: retry once after a short pause
        import time as _time

        _time.sleep(2.0)
        res = run_spmd(in_maps)
    return gather(res.results, bo)
